# revision 26
# baseline (speedup 1.0000x reference)
"""ContextNet dynamic-conv kernel for 8 TRN2 NeuronCores.

Math: the reference computes, per sample b:
    gap[b]  = x[b].sum(T) / len[b]                  (C,)
    h[b]    = sigmoid(gap[b] @ w1.T + b1)           (2C,)
    w_dyn[b, co, ci, k] = h[b, 2*co + (ci>=C/2)] * W[co, ci, k]
        where W = w2.reshape(C, C, K)               (static across batch!)
    out[b]  = conv1d(x[b], w_dyn[b], pad=K//2)      (C, T)

So the per-sample conv weight is a batch-independent tensor W scaled by a
per-sample factor S_b[ci, co] = h[b, 2co + (ci>=64)].  We build S_b
on-chip (tiny matmuls broadcast h across partitions), scale the
pre-transposed weights once per sample (bf16), and run the conv as 5
shifted bf16 matmuls (full 128-deep contraction) accumulating in PSUM
per 512-col tile.  x ships to the device as bf16 (the conv consumes bf16
either way) in chunk-contiguous layout [BL, NCHUNK, C, CHUNK] so every
input DMA is one linear read instead of a 4KB-row strided sweep.
h is computed as sigmoid((gap @ w1t + b1/il) * il): the b1/il term rides
a second 1-row matmul into the same PSUM accumulator, so the whole
affine+sigmoid collapses into one ACT op with a per-sample scale.
Output leaves the device as bf16; the host widens to f32.

Engine plan (Tile's scheduler hoists whatever is ready; bulky elastic
work must never sit in front of a latency-critical chain on the same
queue; GPSIMD cannot touch PSUM and its DMA ring is a *software* DGE —
never put bulk transfers there; ACT is the efficient PSUM drainer):
  Tensor: conv matmuls + tiny h/b1/S matmuls
  ACT   : PSUM->SBUF drains, sigmoid, S->SBUF copy, sample-0 row-sum help
  DVE   : steady row-sum reduces (no chain ops -> nothing blocks them)
  Pool  : gap combine+cast, weight scaling (from SBUF S), halo memsets
  rings : sync HWDGE carries all steady x chunks (FIFO keeps sample
          order) + the tail DMAs; scalar HWDGE carries sample-0's other
          half, the consts, and all output DMAs

Sharding: pure data parallel over batch B=32 -> 4 samples per core x 8.
"""

import numpy as np
from contextlib import ExitStack

import concourse.bacc as bacc
import concourse.tile as tile
from concourse import mybir
from concourse.bass_utils import run_bass_kernel_spmd

B, C, T = 32, 128, 8192
K = 5
PAD = (K - 1) // 2
NCORES = 8
BL = B // NCORES          # samples per core
CHUNK = 2048              # input DMA chunk width (contiguous 512KB blocks)
NCHUNK = T // CHUNK
HC = CHUNK // 2           # row-sum granule: half chunk per engine
TT = 512                  # conv tile width (one PSUM bank of f32)
NTILES = T // TT
OUT_GROUP = 4             # conv tiles batched per output DMA

FP32 = mybir.dt.float32
BF16 = mybir.dt.bfloat16

AF = mybir.ActivationFunctionType
ALU = mybir.AluOpType
AXL = mybir.AxisListType


def build_nc():
    nc = bacc.Bacc("TRN2", target_bir_lowering=False, debug=False)

    x_d = nc.dram_tensor("x", [BL, C, T], BF16, kind="ExternalInput").ap()
    x0c_d = nc.dram_tensor("x0c", [NCHUNK, C, CHUNK], BF16, kind="ExternalInput").ap()
    il_d = nc.dram_tensor("invlen", [1, BL], FP32, kind="ExternalInput").ap()
    w1t_d = nc.dram_tensor("w1t", [C, 2 * C], BF16, kind="ExternalInput").ap()
    b1il_d = nc.dram_tensor("b1il", [1, BL * 2 * C], BF16, kind="ExternalInput").ap()
    wt_d = nc.dram_tensor("wt", [C, K * C], FP32, kind="ExternalInput").ap()
    ones_d = nc.dram_tensor("ones", [1, 64], BF16, kind="ExternalInput").ap()
    out_d = nc.dram_tensor("out", [BL, C, T], BF16, kind="ExternalOutput").ap()

    with ExitStack() as ctx:
        tc = ctx.enter_context(tile.TileContext(nc))

        const = ctx.enter_context(tc.tile_pool(name="const", bufs=1))
        # bufs=4 holds every sample at once: no DMA ever waits on a conv
        # to release a buffer
        xb = ctx.enter_context(tc.tile_pool(name="xb", bufs=4))
        wscp = ctx.enter_context(tc.tile_pool(name="wscp", bufs=2))
        outp = ctx.enter_context(tc.tile_pool(name="outp", bufs=3))
        small = ctx.enter_context(tc.tile_pool(name="small", bufs=3))
        pconv = ctx.enter_context(tc.tile_pool(name="pconv", bufs=3, space="PSUM"))
        ps = ctx.enter_context(tc.tile_pool(name="ps", bufs=1, space="PSUM"))
        ph = ctx.enter_context(tc.tile_pool(name="ph", bufs=1, space="PSUM"))

        wt_sb = const.tile([C, K * C], FP32)
        w1t_sb = const.tile([C, 2 * C], BF16)
        b1il_sb = const.tile([1, BL * 2 * C], BF16)
        il_sb = const.tile([1, BL], FP32)
        ones_sb = const.tile([1, 64], BF16)

        # memset const feeds the sigmoid table preload (must not wait on
        # any DMA)
        sigsrc = const.tile([1, 1], FP32)
        nc.gpsimd.memset(sigsrc[:], 1.0)

        def emit_consts():
            # scalar ring, behind sample 0's x chunks: they land well
            # before the h matmul needs them
            nc.scalar.dma_start(w1t_sb[:], w1t_d[:])
            nc.scalar.dma_start(wt_sb[:], wt_d[:])
            nc.scalar.dma_start(b1il_sb[:], b1il_d[:])
            nc.scalar.dma_start(il_sb[:], il_d[:])
            nc.scalar.dma_start(ones_sb[:], ones_d[:])
            # preload the sigmoid table while ACT is otherwise idle
            sig_d = small.tile([1, 1], FP32, tag="sigd")
            nc.scalar.activation(sig_d[:], sigsrc[:], AF.Sigmoid)

        def emit_load(b):
            """DMA x[b] (bf16) into the halo tile.  Sample 0 reads the
            chunk-contiguous staging copy, split across both HWDGE rings,
            and row-sums in half-chunk granules on DVE+ACT in parallel
            (its load is the kernel's critical path).  Steady samples read
            the natural layout as two partition-split transfers (16KB
            contiguous rows stream ~2.5x faster than 4KB rows) on the sync
            ring alone, and reduce on DVE alone."""
            x_b = xb.tile([C, T + 2 * PAD], BF16)
            nc.gpsimd.memset(x_b[:, 0:PAD], 0.0)
            nc.gpsimd.memset(x_b[:, T + PAD : T + 2 * PAD], 0.0)
            if b == 0:
                # consts first: their issues must not inherit reused DMA
                # semaphores gated on x-chunk landings
                emit_consts()
                for c in range(NCHUNK):
                    lo, hi = c * CHUNK, (c + 1) * CHUNK
                    ring = nc.scalar if c % 2 == 1 else nc.sync
                    ring.dma_start(x_b[:, PAD + lo : PAD + hi], x0c_d[c])
                gap_parts = small.tile([C, 2 * NCHUNK], FP32, tag="gapp0")
                for c in range(NCHUNK):
                    lo = c * CHUNK
                    nc.vector.tensor_reduce(
                        gap_parts[:, 2 * c : 2 * c + 1],
                        x_b[:, PAD + lo : PAD + lo + HC],
                        axis=AXL.X,
                        op=ALU.add,
                    )
                    scratch = small.tile([C, HC], BF16, tag="redscr")
                    nc.scalar.activation(
                        scratch[:],
                        x_b[:, PAD + lo + HC : PAD + lo + CHUNK],
                        AF.Copy,
                        accum_out=gap_parts[:, 2 * c + 1 : 2 * c + 2],
                    )
                g4 = small.tile([C, 4], FP32, tag="g4")
                nc.gpsimd.tensor_tensor(
                    g4[:], gap_parts[:, 0:4], gap_parts[:, 4:8], op=ALU.add
                )
            else:
                nc.sync.dma_start(x_b[0:64, PAD : PAD + T], x_d[b, 0:64, :])
                nc.sync.dma_start(x_b[64:C, PAD : PAD + T], x_d[b, 64:C, :])
                gap_parts = small.tile([C, NCHUNK], FP32, tag="gapp")
                for c in range(NCHUNK):
                    lo, hi = c * CHUNK, (c + 1) * CHUNK
                    nc.vector.tensor_reduce(
                        gap_parts[:, c : c + 1],
                        x_b[:, PAD + lo : PAD + hi],
                        axis=AXL.X,
                        op=ALU.add,
                    )
                g4 = gap_parts
            # combine partials + cast on Pool (no bulk work lives there)
            g2 = small.tile([C, 2], FP32, tag="g2")
            nc.gpsimd.tensor_tensor(g2[:], g4[:, 0:2], g4[:, 2:4], op=ALU.add)
            gap_r = small.tile([C, 1], FP32, tag="gapr")
            nc.gpsimd.tensor_tensor(gap_r[:], g2[:, 0:1], g2[:, 1:2], op=ALU.add)
            gap_bf = small.tile([C, 1], BF16, tag="gapbf")
            nc.gpsimd.tensor_copy(gap_bf[:], gap_r[:])
            return x_b, gap_bf

        def emit_weights(b, gap_bf):
            """h = sigmoid((gap @ w1t + b1/il) * il); S broadcast; scale W."""
            h_ps = ph.tile([1, 2 * C], FP32)
            nc.tensor.matmul(
                h_ps[:], lhsT=gap_bf[:], rhs=w1t_sb[:], start=True, stop=False
            )
            nc.tensor.matmul(
                h_ps[:],
                lhsT=ones_sb[0:1, 0:1],
                rhs=b1il_sb[0:1, b * 2 * C : (b + 1) * 2 * C],
                start=False,
                stop=True,
            )
            h_sb = small.tile([1, 2 * C], BF16, tag="h")
            nc.scalar.activation(
                h_sb[:], h_ps[:], AF.Sigmoid, scale=il_sb[0:1, b : b + 1]
            )

            # S_b[ci, co] = h[2co + (ci>=64)] via contract-1 broadcast
            h3 = h_sb[:].rearrange("p (a two) -> p two a", two=2)  # (1, 2, 128)
            s_ps = ps.tile([C, C], FP32, tag="s")
            nc.tensor.matmul(
                s_ps[0:64, :], lhsT=ones_sb[:], rhs=h3[:, 0, :], start=True, stop=True
            )
            nc.tensor.matmul(
                s_ps[64:128, :], lhsT=ones_sb[:], rhs=h3[:, 1, :], start=True, stop=True
            )
            # S leaves PSUM through ACT so the scaling runs on Pool, whose
            # queue carries no bulk work that could delay it
            s_sb = small.tile([C, C], FP32, tag="ssb")
            nc.scalar.copy(s_sb[:], s_ps[:])
            wsc = wscp.tile([C, K * C], BF16)
            for k in range(K):
                nc.gpsimd.tensor_tensor(
                    wsc[:, k * C : (k + 1) * C],
                    wt_sb[:, k * C : (k + 1) * C],
                    s_sb[:],
                    op=ALU.mult,
                )
            return wsc

        def emit_conv(b, x_b, wsc):
            """5 shifted matmuls per 512-tile; ACT drains 2 PSUM banks per
            copy; Pool issues the output DMAs.  The last sample streams its
            output at pair granularity (tile granularity in the final
            group, on two engines + the idle sync ring) so the kernel tail
            isn't gated on one large copy+DMA."""
            last = b == BL - 1
            for g in range(NTILES // OUT_GROUP):
                lastg = last and g == NTILES // OUT_GROUP - 1
                o_sb = outp.tile([C, OUT_GROUP * TT], BF16)
                for jj in range(0, OUT_GROUP, 2):
                    pc = pconv.tile([C, 2 * TT], FP32)
                    for half in range(2):
                        j = g * OUT_GROUP + jj + half
                        for k in range(K):
                            nc.tensor.matmul(
                                pc[:, half * TT : (half + 1) * TT],
                                lhsT=wsc[:, k * C : (k + 1) * C],
                                rhs=x_b[:, j * TT + k : j * TT + k + TT],
                                start=(k == 0),
                                stop=(k == K - 1),
                            )
                        if lastg:
                            hs = slice(half * TT, (half + 1) * TT)
                            osl = o_sb[:, (jj + half) * TT : (jj + half + 1) * TT]
                            if half == 0:
                                nc.scalar.copy(osl, pc[:, hs])
                            else:
                                nc.vector.tensor_copy(osl, pc[:, hs])
                            col0 = (g * OUT_GROUP + jj + half) * TT
                            nc.sync.dma_start(out_d[b, :, col0 : col0 + TT], osl)
                    if not lastg:
                        nc.scalar.copy(o_sb[:, jj * TT : (jj + 2) * TT], pc[:])
                        if last:
                            col0 = (g * OUT_GROUP + jj) * TT
                            nc.scalar.dma_start(
                                out_d[b, :, col0 : col0 + 2 * TT],
                                o_sb[:, jj * TT : (jj + 2) * TT],
                            )
                if not lastg and not last:
                    nc.scalar.dma_start(
                        out_d[b, :, g * OUT_GROUP * TT : (g + 1) * OUT_GROUP * TT],
                        o_sb[:],
                    )

        # Software pipeline, one sample deep: emit load(b+1) AND weights(b+1)
        # before conv(b) so the b+1 weight chain interleaves into conv(b)
        # as soon as sample b+1's data lands.
        x_b_cur, gap_cur = emit_load(0)
        wsc_cur = emit_weights(0, gap_cur)
        for b in range(BL):
            nxt = None
            if b + 1 < BL:
                x_b_n, gap_n = emit_load(b + 1)
                wsc_n = emit_weights(b + 1, gap_n)
                nxt = (x_b_n, wsc_n)
            emit_conv(b, x_b_cur, wsc_cur)
            if nxt is not None:
                x_b_cur, wsc_cur = nxt

    nc.compile()
    return nc


_NC_CACHE = None


def _get_nc():
    global _NC_CACHE
    if _NC_CACHE is None:
        _NC_CACHE = build_nc()
    return _NC_CACHE


def make_in_maps(x, input_lengths, w1, b1, w2):
    import ml_dtypes

    x = np.asarray(x, dtype=np.float32).astype(ml_dtypes.bfloat16)
    lens = np.asarray(input_lengths).astype(np.float64)
    invlen = (1.0 / lens).astype(np.float32)
    # b1/il per sample: rides a second matmul into the h accumulator so
    # sigmoid((gap@w1t + b1/il)*il) == sigmoid(gap@w1t*il + b1) exactly
    b1il = (np.asarray(b1, dtype=np.float64)[None, :] * lens[:, None]).astype(
        ml_dtypes.bfloat16
    )  # (B, 2C)
    w1t = np.ascontiguousarray(
        np.asarray(w1, dtype=np.float32).T.astype(ml_dtypes.bfloat16)
    )  # (C, 2C) bf16
    # wt[ci, k*C + co] = W[co, ci, k],  W = w2.reshape(C, C, K)
    wt = np.ascontiguousarray(
        np.asarray(w2, dtype=np.float32)
        .reshape(C, C, K)
        .transpose(1, 2, 0)
        .reshape(C, K * C)
    )
    ones = np.ones((1, 64), dtype=ml_dtypes.bfloat16)

    in_maps = []
    for i in range(NCORES):
        sl = slice(i * BL, (i + 1) * BL)
        # chunk-contiguous staging copy of this core's sample 0 only
        x0c = x[sl][0].reshape(C, NCHUNK, CHUNK).transpose(1, 0, 2)
        in_maps.append(
            {
                "x": np.ascontiguousarray(x[sl]),
                "x0c": np.ascontiguousarray(x0c),
                "invlen": np.ascontiguousarray(invlen[sl].reshape(1, BL)),
                "w1t": w1t,
                "b1il": np.ascontiguousarray(b1il[sl].reshape(1, BL * 2 * C)),
                "wt": wt,
                "ones": ones,
            }
        )
    return in_maps


def kernel(x, input_lengths, w1, b1, w2, _trace=False):
    nc = _get_nc()
    in_maps = make_in_maps(x, input_lengths, w1, b1, w2)
    res = run_bass_kernel_spmd(nc, in_maps, core_ids=list(range(NCORES)), trace=_trace)
    out = np.concatenate(
        [res.results[i]["out"].astype(np.float32) for i in range(NCORES)], axis=0
    )
    if _trace:
        kernel.last_exec_time_ns = res.exec_time_ns
        kernel.last_results = res
    return out
